# revision 78
# baseline (speedup 1.0000x reference)
"""Trainium2 Bass kernel for a local-window multi-head attention block.

Math (per batch element b, all in one NeuronCore; batch is data-parallel
across the 8 cores):
    qkv  = x @ w_qkv.T                      [N, 2304]
    q,k,v split into 12 heads of dim 64, q scaled by 1/8
    S    = q @ k.T + local mask             (mask: |dh|<=3, |dw|<=5 on a 16x64 grid)
    P    = softmax(S); O = P @ v
    out  = O @ w_proj.T + b_proj

Device layout notes:
  - Tokens are permuted on the host to w-major order (n' = 16*w + h).
    The local window |dw|<=5 then spans only 11 of 64 w-columns, so each
    128-key chunk's visible queries fit in a 288-wide contiguous window.
  - Everything is computed transposed (channels on partitions):
    qkvT = w_qkv @ x.T via PE, S^T tiles per 128-key chunk over a
    288-wide query window, softmax without max-subtraction (scores are
    tiny), row sums via an appended ones-column in the P@V matmul.
  - x arrives in four 256-token blocks so the first q-projection matmul
    can start as soon as the first 192KB lands.
  - The P@V for the odd head of each pair uses a 128-wide V slice (96
    cols per head slot) that places the ones-column at slice position
    32, so its O^T lands on psum partitions 64:128 (row 32 = the
    denominator, 32-aligned for PSUM reads) and the normalize writes
    OT_sb[64:128] directly - no staging DMA.
  - Softmax denominators: both heads' r rows stage to partition 0 side
    by side, one fast reciprocal each, GPSIMD partition-broadcast
    (whose output must start at partition 0), one multiply per head.
  - All matmuls run in bf16 (fp8 DoubleRow for the k projection was
    measured correct via [P, 2, N] j-plane operands but net-slower:
    the schedule is chain-limited, and shrinking the projection work
    just removes the filler that hides the softmax chains).
  - Attention chains are padded with a filler queue of independent PE
    work (next pair's q/k projection, v chunks, and both output
    projection partials inside the last pair) pulled between S and P@V
    groups so the PE never idles and the HAM clock gate stays at
    2.4 GHz. Fillers must never depend on the current pair's output -
    the PE queue is in-order, so a blocked filler stalls everything.
  - Sub-region psum accumulation groups must be emitted region-major
    (region loop outer); interleaving them corrupts the accumulation.
"""

import os
import sys

sys.path.insert(0, "/opt/trn_rl_repo")

import numpy as np

B, N, DIM = 8, 1024, 768
NH, HD = 12, 64
SCALE = HD ** -0.5
P = 128
CN = DIM // P            # 6 channel chunks
MC = N // P              # 8 token chunks
TWIN = 512               # output/projection tile width (1 psum bank)
NT = N // TWIN           # 2 output tiles
XB = 256                 # x DMA block (tokens)
NXB = N // XB            # 4 x blocks
AWIN = 288               # attention query window per 128-key chunk (w-major)
WIN_START = [min(max(128 * c - 80, 0), N - AWIN) for c in range(MC)]

# w-major permutation: new token n' = 16*w + h  ->  original n = 64*h + w
PERM = np.array([64 * (i % 16) + (i // 16) for i in range(N)])

# host weight layout offsets (all contiguous per partition):
#   [p, hp, qk, co, 128] for q/k: 6 pairs * 2 * 6 * 128 = 9216
#   [p, oh, co, 384] for v: 2 * 6 * 384 = 4608
WQK_SZ = 2 * CN * P      # 1536 per (hp)
WV_OFF = CN * WQK_SZ     # 9216
WQKV_COLS = WV_OFF + 2 * CN * 384  # 13824

_PROG = None


def _emit(ctx, tc, aps, debug=None):
    import concourse.bass as bass
    import concourse.mybir as mybir

    nc = tc.nc
    f32 = mybir.dt.float32
    bf16 = mybir.dt.bfloat16
    AF = mybir.ActivationFunctionType
    add = mybir.AluOpType.add

    xT, wqkvT, wprojT, biasT, bmask, ident, outT = aps

    consts = ctx.enter_context(tc.tile_pool(name="consts", bufs=1))
    wpool = ctx.enter_context(tc.tile_pool(name="wstream", bufs=4))
    psum = ctx.enter_context(tc.tile_pool(name="ps", bufs=2, space="PSUM"))
    spool_box = {}
    etpool = ctx.enter_context(tc.tile_pool(name="etp", bufs=4))
    rpool = ctx.enter_context(tc.tile_pool(name="rp", bufs=3))
    bpool = ctx.enter_context(tc.tile_pool(name="invbp", bufs=4))
    expool = ctx.enter_context(tc.tile_pool(name="exp_scratch", bufs=4))

    def load_wqk(hp, split=False):
        """Fetch the bf16 q/k weight chunks for head pair hp (one DMA)."""
        base = 2 * hp * CN * P
        w = wpool.tile([P, 2, CN, P], bf16, name="wqk%d" % (hp % 2), tag="wqk")
        src = wqkvT[:, base: base + 2 * CN * P].rearrange(
            "p (q c o) -> p q c o", q=2, c=CN)
        if split:
            # first pair: q half first so the very first matmul can start
            nc.sync.dma_start(w[:, 0], src[:, 0])
            nc.sync.dma_start(w[:, 1], src[:, 1])
        else:
            nc.sync.dma_start(w[:], src)
        return w

    # Startup order: q weights of pair 0, then x block 0 (the first psum
    # accumulation needs only those), then the rest of x, then v/proj
    # weights.
    xT_r = xT.rearrange("p (b c n) -> p b c n", b=NXB, c=CN)
    xT_sb = consts.tile([P, NXB, CN, XB], bf16)
    wqk0 = load_wqk(0, split=True)
    for b in range(NXB):
        nc.sync.dma_start(xT_sb[:, b], xT_r[:, b])
    wv = consts.tile([P, 2, CN, 384], bf16)
    nc.sync.dma_start(
        wv[:], wqkvT[:, WV_OFF:].rearrange("p (h c o) -> p h c o", h=2, c=CN))
    # proj weights early: they only need DMA bandwidth, and loading them
    # here removes the PE stall before the output projection.
    wprojT_sb = consts.tile([P, CN, DIM], bf16)
    nc.sync.dma_start(
        wprojT_sb[:], wprojT.rearrange("p (c o) -> p c o", c=CN))
    bias_sb = consts.tile([P, CN], f32)
    nc.sync.dma_start(bias_sb[:], biasT[:])
    bmask_sb = consts.tile([P, MC, AWIN], bf16)
    nc.sync.dma_start(bmask_sb[:], bmask[:])
    I_sb = consts.tile([P, P], bf16)
    nc.sync.dma_start(I_sb[:], ident[:])

    qkT_sb = consts.tile([P, 2 * CN, N], bf16)     # chunks 0..5 = q, 6..11 = k
    # V slots are 96 wide [v(64), ones, pad(31)] so the odd head's 128-wide
    # slice (start 96*h1-64) puts the ones column at slice position 32 -> its
    # row sum lands on psum partition 32 (PSUM reads must be 32-aligned).
    VW = 96
    V_sb = consts.tile([P, MC, NH + 1, VW], bf16)  # col 64 = ones
    V_flat = V_sb.rearrange("p m h c -> p m (h c)")
    OT_sb = consts.tile([P, CN, N], bf16)
    nc.gpsimd.memset(V_sb[:], 0.0)
    nc.vector.memset(V_sb[:, :, :, 64:65], 1.0)
    if debug is not None:
        debug.update(qkT_sb=qkT_sb, V_sb=V_sb, OT_sb=OT_sb)

    def emit_qk(hp, w, qk, t, by_block, wsel=None):
        """One bf16 q or k projection psum tile: [128 out, TWIN tokens]."""
        qps = psum.tile([P, TWIN], f32, name="qps", tag="qkv")
        # psum sub-region accumulation groups must not interleave in
        # emission order: keep the region (j) loop OUTER
        if by_block:
            for j in range(2):
                for k in range(CN):
                    lhsT = w[:, k, :] if wsel is None else w[:, wsel, k, :]
                    nc.tensor.matmul(
                        qps[:, XB * j: XB * (j + 1)],
                        lhsT=lhsT,
                        rhs=xT_sb[:, 2 * t + j, k, :],
                        start=(k == 0),
                        stop=(k == CN - 1),
                    )
        else:
            for k in range(CN):
                lhsT = w[:, k, :] if wsel is None else w[:, wsel, k, :]
                nc.tensor.matmul(
                    qps[:],
                    lhsT=lhsT,
                    rhs=xT_sb[:, 2 * t: 2 * t + 2, k, :],
                    start=(k == 0),
                    stop=(k == CN - 1),
                )
        dst = qkT_sb[:, CN * qk + hp, TWIN * t: TWIN * (t + 1)]
        if (qk + t) % 2 == 0:
            nc.vector.tensor_copy(dst, qps[:])
        else:
            nc.scalar.activation(dst, qps[:], AF.Copy)

    def emit_v(oh, m):
        """One v projection chunk: psum [128 tokens, 384 outs]."""
        vps = psum.tile([P, TWIN], f32, name="vps", tag="qkv")
        for k in range(CN):
            nc.tensor.matmul(
                vps[:, 0:384],
                lhsT=xT_sb[:, m // 2, k, P * (m % 2): P * (m % 2) + P],
                rhs=wv[:, oh, k, :],
                start=(k == 0),
                stop=(k == CN - 1),
            )
        nc.scalar.activation(
            V_sb[:, m, 6 * oh: 6 * (oh + 1), 0:64],
            vps[:, 0:384].rearrange("p (a b) -> p a b", b=64),
            AF.Copy,
        )

    def pull(filler):
        if filler:
            filler.pop(0)()

    def emit_attention(hp, filler):
        """S, softmax and P@V for head pair hp (both heads interleaved).

        filler: list of thunks emitting independent PE work; one is
        pulled after each S chunk-pair and each P@V group so the PE has
        fill work while the exp/mask chains run.
        """
        ets = [etpool.tile([P, MC, AWIN], bf16, name="et%d" % hh, tag="et")
               for hh in range(2)]
        for cp in range(MC // 2):
            spss = [spool_box["p"].tile([P, 2, TWIN], f32, name="sps%d" % hh,
                                        tag="sps")
                    for hh in range(2)]
            for j in range(2):
                c = 2 * cp + j
                s = WIN_START[c]
                for hh in range(2):
                    prange = slice(64 * hh, 64 * hh + 64)
                    nc.tensor.matmul(
                        spss[hh][:, j, 0:AWIN],
                        lhsT=qkT_sb[prange, CN + hp, P * c: P * (c + 1)],
                        rhs=qkT_sb[prange, hp, s: s + AWIN],
                        start=True,
                        stop=True,
                        tile_position=(64 * hh, 0),
                    )
            pull(filler)
            for hh in range(2):
                esc = expool.tile([P, 2, AWIN], bf16, name="esc", tag="esc")
                nc.scalar.activation(
                    esc[:], spss[hh][:, :, 0:AWIN], AF.Exp)
                nc.vector.tensor_mul(
                    ets[hh][:, 2 * cp: 2 * cp + 2, :].rearrange(
                        "p a b -> p (a b)"),
                    esc[:].rearrange("p a b -> p (a b)"),
                    bmask_sb[:, 2 * cp: 2 * cp + 2, :].rearrange(
                        "p a b -> p (a b)"),
                )
        if debug is not None:
            for hh in range(2):
                if ("d_et%d" % (2 * hp + hh)) in debug:
                    nc.sync.dma_start(debug["d_et%d" % (2 * hp + hh)][:], ets[hh][:])
        # P@V with ones column. hh=0: lhsT = [v,ones,...] -> O^T on psum
        # rows 0:64, r on row 64. hh=1: lhsT starts 64 cols earlier so
        # O^T lands on rows 64:128 with r on row 32 (the previous head's
        # ones column); the other rows hold garbage that is never read.
        # 128-wide weight loads keep FWL enabled for both.
        for t in range(NT):
            ots = []
            for hh in range(2):
                h = 2 * hp + hh
                et = ets[hh]
                ot = psum.tile([P, TWIN], f32, name="ot%d" % hh, tag="ot")
                ots.append(ot)
                base = VW * h if hh == 0 else VW * h - 64
                cs = [c for c in range(MC)
                      if min(WIN_START[c] + AWIN, TWIN * (t + 1)) > max(WIN_START[c], TWIN * t)]
                # widest-overlap chunk first so the start=True matmul covers
                # the largest psum range (per-element has_written then only
                # ever accumulates into written elements)
                cs.sort(key=lambda c: max(WIN_START[c], TWIN * t)
                        - min(WIN_START[c] + AWIN, TWIN * (t + 1)))
                for i, c in enumerate(cs):
                    lo = max(WIN_START[c], TWIN * t)
                    hi = min(WIN_START[c] + AWIN, TWIN * (t + 1))
                    nc.tensor.matmul(
                        ot[:, lo - TWIN * t: hi - TWIN * t],
                        lhsT=V_flat[:, c, base: base + 128],
                        rhs=et[:, c, lo - WIN_START[c]: hi - WIN_START[c]],
                        start=(i == 0),
                        stop=(i == len(cs) - 1),
                    )
            pull(filler)
            # normalize: both r rows -> reciprocal -> per-head broadcast ->
            # per-head multiply straight into OT_sb
            # stage the r rows to SBUF (reciprocal_approx_fast misreads
            # PSUM operands on HW), per-head fast reciprocal, then GPSIMD
            # partition broadcast (output must start at partition 0)
            rr = rpool.tile([1, 2, TWIN], f32, name="rr", tag="rr")
            nc.scalar.activation(rr[:, 0, :], ots[0][64:65, :], AF.Copy)
            nc.vector.tensor_copy(rr[:, 1, :], ots[1][32:33, :])
            dst0 = OT_sb[0:64, hp, TWIN * t: TWIN * (t + 1)]
            dst1 = OT_sb[64:128, hp, TWIN * t: TWIN * (t + 1)]
            invr = rpool.tile([1, 2, TWIN], f32, name="invr", tag="invr")
            invb0 = bpool.tile([P, TWIN], f32, name="invb0", tag="invb")
            invb1 = bpool.tile([P, TWIN], f32, name="invb1", tag="invb")
            nc.vector.reciprocal_approx_fast(invr[:, 0, :], rr[:, 0, :])
            nc.gpsimd.partition_broadcast(invb0[:, :], invr[:, 0, :])
            nc.vector.reciprocal_approx_fast(invr[:, 1, :], rr[:, 1, :])
            nc.gpsimd.partition_broadcast(invb1[:, :], invr[:, 1, :])
            nc.vector.tensor_mul(dst0, ots[0][0:64, :], invb0[0:64, :])
            nc.vector.tensor_mul(dst1, ots[1][64:128, :], invb1[64:128, :])

    # Output projection partials: k-chunks accumulate into an SBUF
    # partial as soon as the corresponding attention pairs finish; the
    # tail folds the remaining chunks + bias + partial per (oc, t).
    # Partials only use chunks 0..3 so they are safe fillers one slot
    # later (a filler must never depend on the current pair's output -
    # it would stall the in-order PE queue).
    KSPLIT = {0: 4, 1: 4}
    partial_sb = consts.tile([P, CN, N], bf16)

    def emit_proj_part(t, oc):
        pps = psum.tile([P, TWIN], f32, name="pps", tag="qkv")
        for k in range(KSPLIT[t]):
            nc.tensor.matmul(
                pps[:],
                lhsT=wprojT_sb[:, k, P * oc: P * (oc + 1)],
                rhs=OT_sb[:, k, TWIN * t: TWIN * (t + 1)],
                start=(k == 0),
                stop=(k == KSPLIT[t] - 1),
            )
        dst = partial_sb[:, oc, TWIN * t: TWIN * (t + 1)]
        if oc % 2 == 0:
            nc.vector.tensor_copy(dst, pps[:])
        else:
            nc.scalar.activation(dst, pps[:], AF.Copy)

    # Attention runs pairs in order [0,1,2,3,5,4] so both projection
    # partials (k-chunks 0..3) become safe fillers for the last two
    # positions. Fillers per position: next pair's q (bf16) + k (fp8)
    # projections, v head-triples (g2 = heads 6-8 before position 3,
    # g3 = heads 9-11 before position 4), then the partials.
    ATT_ORDER = [0, 1, 2, 3, 4, 5]
    V1_MS = {0: [0, 1, 2], 1: [3, 4, 5], 2: [6, 7]}

    with tc.tile_pool(name="spsp", bufs=2, space="PSUM") as spool:
        spool_box["p"] = spool
        # pair-0 q/k projection + first-half v: the PE's warm-up block
        # t-outer: the two t=0 units need only x blocks 0,1 (landed
        # first), covering the wait for blocks 2,3
        for t in range(NT):
            for qk in range(2):
                emit_qk(0, wqk0, qk, t, by_block=True, wsel=qk)
        for m in range(MC):
            emit_v(0, m)
        wq_next = load_wqk(1)
        for pos in range(CN):
            hp = ATT_ORDER[pos]
            wq_cur = wq_next
            filler = []
            if pos + 1 < CN:
                hpn = ATT_ORDER[pos + 1]
                if pos + 2 < CN:
                    wq_next = load_wqk(ATT_ORDER[pos + 2])
                for qk in range(2):
                    for t in range(NT):
                        filler.append(
                            lambda hpn=hpn, qk=qk, t=t, w=wq_cur: emit_qk(
                                hpn, w, qk, t, by_block=False, wsel=qk))
            if pos in V1_MS:
                for m in V1_MS[pos]:
                    filler.append(lambda m=m: emit_v(1, m))
            if pos == 5:
                # both partials fill the last pair's softmax chains; the
                # leftovers bridge the window between the last P@V and
                # the projection tail
                for oc in range(CN):
                    filler.append(lambda oc=oc: emit_proj_part(0, oc))
                    filler.append(lambda oc=oc: emit_proj_part(1, oc))
            emit_attention(hp, filler)
            for f in filler:
                f()

    # ---------------- output projection tail ----------------
    # t=0 first: its OT chunks complete before t=1's (the t=1 chains of
    # the last pair are the final attention work)
    with tc.tile_pool(name="outst", bufs=4) as ostpool, \
            tc.tile_pool(name="tailp", bufs=4, space="PSUM") as tpool:
        # Per t, issue the k=4 matmuls (which depend only on pair 4's
        # output) for as many ocs as psum buffers allow BEFORE any k=5
        # matmul: a k=5 waits on the last pair's softmax chain, and in
        # the in-order PE queue it would otherwise block the independent
        # k=4 work behind it.
        PRE = 4
        for t in range(NT):
            def k4(oc, t=t):
                pps = tpool.tile([P, TWIN], f32, name="pps", tag="tpps")
                nc.tensor.matmul(
                    pps[:],
                    lhsT=wprojT_sb[:, 4, P * oc: P * (oc + 1)],
                    rhs=OT_sb[:, 4, TWIN * t: TWIN * (t + 1)],
                    start=True,
                    stop=False,
                )
                return pps
            pend = {oc: k4(oc) for oc in range(PRE)}
            for oc in range(CN):
                pps = pend.pop(oc)
                nc.tensor.matmul(
                    pps[:],
                    lhsT=wprojT_sb[:, 5, P * oc: P * (oc + 1)],
                    rhs=OT_sb[:, 5, TWIN * t: TWIN * (t + 1)],
                    start=False,
                    stop=True,
                )
                ost = ostpool.tile([P, TWIN], bf16, name="ost", tag="ost")
                # ost = (pps + bias) + partial in one DVE op
                nc.vector.scalar_tensor_tensor(
                    ost[:], pps[:], bias_sb[:, oc: oc + 1],
                    partial_sb[:, oc, TWIN * t: TWIN * (t + 1)],
                    add, add)
                nc.sync.dma_start(
                    outT[P * oc: P * (oc + 1), TWIN * t: TWIN * (t + 1)], ost[:])
                if PRE + oc < CN:
                    pend[PRE + oc] = k4(PRE + oc)


def _build(debug_shapes=False):
    global _PROG
    if _PROG is not None:
        return _PROG
    from contextlib import ExitStack

    from concourse import bacc
    import concourse.mybir as mybir
    import concourse.tile as tile

    f32 = mybir.dt.float32
    bf16 = mybir.dt.bfloat16

    nc = bacc.Bacc("TRN2", target_bir_lowering=False, debug=False,
                   enable_asserts=False)
    xT = nc.dram_tensor("xT", [P, CN * N], bf16, kind="ExternalInput").ap()
    wqkvT = nc.dram_tensor("wqkvT", [P, WQKV_COLS], bf16, kind="ExternalInput").ap()
    wprojT = nc.dram_tensor("wprojT", [P, CN * DIM], bf16, kind="ExternalInput").ap()
    biasT = nc.dram_tensor("biasT", [P, CN], f32, kind="ExternalInput").ap()
    bmask = nc.dram_tensor("bmask", [P, MC, AWIN], bf16, kind="ExternalInput").ap()
    ident = nc.dram_tensor("ident", [P, P], bf16, kind="ExternalInput").ap()
    outT = nc.dram_tensor("outT", [DIM, N], bf16, kind="ExternalOutput").ap()

    with tile.TileContext(nc) as tc:
        with ExitStack() as ctx:
            _emit(ctx, tc, (xT, wqkvT, wprojT, biasT, bmask, ident, outT))
    nc.compile()
    _PROG = nc
    return nc


def _host_inputs(x, w_qkv, w_proj, b_proj, mask):
    import ml_dtypes

    x = np.asarray(x, dtype=np.float32)
    w_qkv = np.asarray(w_qkv, dtype=np.float32)
    w_proj = np.asarray(w_proj, dtype=np.float32)
    b_proj = np.asarray(b_proj, dtype=np.float32)
    mask = np.asarray(mask, dtype=np.float32)

    wq = w_qkv.copy()
    wq[0:DIM] *= SCALE
    wT = np.ascontiguousarray(wq.T)                          # [768 in, 2304 out]
    # q/k blocks: [p, hp, qk, co, 128] ; v blocks: [p, oh, co, 384]
    wqkv_host = np.empty((P, WQKV_COLS), dtype=np.float32)
    for hp in range(CN):
        for qk in range(2):
            blk = wT[:, DIM * qk + P * hp: DIM * qk + P * hp + P]  # [768, 128]
            blk = blk.reshape(CN, P, P).transpose(1, 0, 2).reshape(P, CN * P)
            base = (2 * hp + qk) * CN * P
            wqkv_host[:, base: base + CN * P] = blk
    for oh in range(2):
        blk = wT[:, 2 * DIM + 384 * oh: 2 * DIM + 384 * (oh + 1)]  # [768, 384]
        blk = blk.reshape(CN, P, 384).transpose(1, 0, 2).reshape(P, CN * 384)
        wqkv_host[:, WV_OFF + oh * CN * 384: WV_OFF + (oh + 1) * CN * 384] = blk
    wqkvT = wqkv_host.astype(ml_dtypes.bfloat16)

    wprojT = np.ascontiguousarray(
        w_proj.T.reshape(CN, P, DIM).transpose(1, 0, 2).reshape(P, CN * DIM)
    ).astype(ml_dtypes.bfloat16)
    biasT = np.ascontiguousarray(b_proj.reshape(CN, P).T)    # [128, 6]

    vis = (mask[0, 0] == 0.0)
    vis_w = vis[np.ix_(PERM, PERM)]
    bm = np.zeros((P, MC, AWIN), dtype=ml_dtypes.bfloat16)
    for c in range(MC):
        s = WIN_START[c]
        bm[:, c, :] = vis_w[c * P:(c + 1) * P, s: s + AWIN]

    in_maps = []
    for b in range(B):
        xw = np.ascontiguousarray(x[b].T[:, PERM])           # [768, 1024] w-major
        # bf16: [p, block, c, 256]
        xTb = (xw.reshape(CN, P, NXB, XB).transpose(1, 2, 0, 3)
               .reshape(P, CN * N))
        in_maps.append({
            "xT": xTb.astype(ml_dtypes.bfloat16),
            "wqkvT": wqkvT,
            "wprojT": wprojT,
            "biasT": biasT,
            "bmask": bm,
            "ident": np.eye(P, dtype=ml_dtypes.bfloat16),
        })
    return in_maps


PROFILE = False
LAST_RESULT = None


def kernel(x, w_qkv, w_proj, b_proj, mask):
    global LAST_RESULT
    from concourse.bass_utils import run_bass_kernel_spmd

    nc = _build()
    in_maps = _host_inputs(x, w_qkv, w_proj, b_proj, mask)
    res = run_bass_kernel_spmd(nc, in_maps, core_ids=list(range(B)),
                               trace=PROFILE)
    LAST_RESULT = res
    out = np.empty((B, N, DIM), dtype=np.float32)
    for b in range(B):
        out[b][PERM, :] = np.asarray(res.results[b]["outT"]).astype(np.float32).T
    return np.ascontiguousarray(out)


# revision 79
# speedup vs baseline: 1.0024x; 1.0024x over previous
"""Trainium2 Bass kernel for a local-window multi-head attention block.

Math (per batch element b, all in one NeuronCore; batch is data-parallel
across the 8 cores):
    qkv  = x @ w_qkv.T                      [N, 2304]
    q,k,v split into 12 heads of dim 64, q scaled by 1/8
    S    = q @ k.T + local mask             (mask: |dh|<=3, |dw|<=5 on a 16x64 grid)
    P    = softmax(S); O = P @ v
    out  = O @ w_proj.T + b_proj

Device layout notes:
  - Tokens are permuted on the host to w-major order (n' = 16*w + h).
    The local window |dw|<=5 then spans only 11 of 64 w-columns, so each
    128-key chunk's visible queries fit in a 288-wide contiguous window.
  - Everything is computed transposed (channels on partitions):
    qkvT = w_qkv @ x.T via PE, S^T tiles per 128-key chunk over a
    288-wide query window, softmax without max-subtraction (scores are
    tiny), row sums via an appended ones-column in the P@V matmul.
  - x arrives in four 256-token blocks so the first q-projection matmul
    can start as soon as the first 192KB lands.
  - The P@V for the odd head of each pair uses a 128-wide V slice (96
    cols per head slot) that places the ones-column at slice position
    32, so its O^T lands on psum partitions 64:128 (row 32 = the
    denominator, 32-aligned for PSUM reads) and the normalize writes
    OT_sb[64:128] directly - no staging DMA.
  - Softmax denominators: both heads' r rows stage to partition 0 side
    by side, one fast reciprocal each, GPSIMD partition-broadcast
    (whose output must start at partition 0), one multiply per head.
  - All matmuls run in bf16 (fp8 DoubleRow for the k projection was
    measured correct via [P, 2, N] j-plane operands but net-slower:
    the schedule is chain-limited, and shrinking the projection work
    just removes the filler that hides the softmax chains).
  - Attention chains are padded with a filler queue of independent PE
    work (next pair's q/k projection, v chunks, and both output
    projection partials inside the last pair) pulled between S and P@V
    groups so the PE never idles and the HAM clock gate stays at
    2.4 GHz. Fillers must never depend on the current pair's output -
    the PE queue is in-order, so a blocked filler stalls everything.
  - Sub-region psum accumulation groups must be emitted region-major
    (region loop outer); interleaving them corrupts the accumulation.
"""

import os
import sys

sys.path.insert(0, "/opt/trn_rl_repo")

import numpy as np

B, N, DIM = 8, 1024, 768
NH, HD = 12, 64
SCALE = HD ** -0.5
P = 128
CN = DIM // P            # 6 channel chunks
MC = N // P              # 8 token chunks
TWIN = 512               # output/projection tile width (1 psum bank)
NT = N // TWIN           # 2 output tiles
XB = 256                 # x DMA block (tokens)
NXB = N // XB            # 4 x blocks
AWIN = 288               # attention query window per 128-key chunk (w-major)
WIN_START = [min(max(128 * c - 80, 0), N - AWIN) for c in range(MC)]

# w-major permutation: new token n' = 16*w + h  ->  original n = 64*h + w
PERM = np.array([64 * (i % 16) + (i // 16) for i in range(N)])

# host weight layout offsets (all contiguous per partition):
#   [p, hp, qk, co, 128] for q/k: 6 pairs * 2 * 6 * 128 = 9216
#   [p, oh, co, 384] for v: 2 * 6 * 384 = 4608
WQK_SZ = 2 * CN * P      # 1536 per (hp)
WV_OFF = CN * WQK_SZ     # 9216
WQKV_COLS = WV_OFF + 2 * CN * 384  # 13824

_PROG = None


def _emit(ctx, tc, aps, debug=None):
    import concourse.bass as bass
    import concourse.mybir as mybir

    nc = tc.nc
    f32 = mybir.dt.float32
    bf16 = mybir.dt.bfloat16
    AF = mybir.ActivationFunctionType
    add = mybir.AluOpType.add

    xT, wqkvT, wprojT, biasT, bmask, ident, outT = aps

    consts = ctx.enter_context(tc.tile_pool(name="consts", bufs=1))
    wpool = ctx.enter_context(tc.tile_pool(name="wstream", bufs=4))
    psum = ctx.enter_context(tc.tile_pool(name="ps", bufs=2, space="PSUM"))
    spool_box = {}
    etpool = ctx.enter_context(tc.tile_pool(name="etp", bufs=4))
    rpool = ctx.enter_context(tc.tile_pool(name="rp", bufs=3))
    bpool = ctx.enter_context(tc.tile_pool(name="invbp", bufs=4))
    expool = ctx.enter_context(tc.tile_pool(name="exp_scratch", bufs=4))

    def load_wqk(hp, split=False):
        """Fetch the bf16 q/k weight chunks for head pair hp (one DMA)."""
        base = 2 * hp * CN * P
        w = wpool.tile([P, 2, CN, P], bf16, name="wqk%d" % (hp % 2), tag="wqk")
        src = wqkvT[:, base: base + 2 * CN * P].rearrange(
            "p (q c o) -> p q c o", q=2, c=CN)
        if split:
            # first pair: q half first so the very first matmul can start
            nc.sync.dma_start(w[:, 0], src[:, 0])
            nc.sync.dma_start(w[:, 1], src[:, 1])
        else:
            nc.sync.dma_start(w[:], src)
        return w

    # Startup order: q weights of pair 0, then x block 0 (the first psum
    # accumulation needs only those), then the rest of x, then v/proj
    # weights.
    xT_r = xT.rearrange("p (b c n) -> p b c n", b=NXB, c=CN)
    xT_sb = consts.tile([P, NXB, CN, XB], bf16)
    wqk0 = load_wqk(0, split=True)
    for b in range(NXB):
        nc.sync.dma_start(xT_sb[:, b], xT_r[:, b])
    wv = consts.tile([P, 2, CN, 384], bf16)
    nc.sync.dma_start(
        wv[:], wqkvT[:, WV_OFF:].rearrange("p (h c o) -> p h c o", h=2, c=CN))
    # proj weights early: they only need DMA bandwidth, and loading them
    # here removes the PE stall before the output projection.
    wprojT_sb = consts.tile([P, CN, DIM], bf16)
    nc.sync.dma_start(
        wprojT_sb[:], wprojT.rearrange("p (c o) -> p c o", c=CN))
    bias_sb = consts.tile([P, CN], f32)
    nc.sync.dma_start(bias_sb[:], biasT[:])
    bmask_sb = consts.tile([P, MC, AWIN], bf16)
    nc.sync.dma_start(bmask_sb[:], bmask[:])
    I_sb = consts.tile([P, P], bf16)
    nc.sync.dma_start(I_sb[:], ident[:])

    qkT_sb = consts.tile([P, 2 * CN, N], bf16)     # chunks 0..5 = q, 6..11 = k
    # V slots are 96 wide [v(64), ones, pad(31)] so the odd head's 128-wide
    # slice (start 96*h1-64) puts the ones column at slice position 32 -> its
    # row sum lands on psum partition 32 (PSUM reads must be 32-aligned).
    VW = 96
    V_sb = consts.tile([P, MC, NH + 1, VW], bf16)  # col 64 = ones
    V_flat = V_sb.rearrange("p m h c -> p m (h c)")
    OT_sb = consts.tile([P, CN, N], bf16)
    nc.gpsimd.memset(V_sb[:], 0.0)
    nc.vector.memset(V_sb[:, :, :, 64:65], 1.0)
    if debug is not None:
        debug.update(qkT_sb=qkT_sb, V_sb=V_sb, OT_sb=OT_sb)

    def emit_qk(hp, w, qk, t, by_block, wsel=None):
        """One bf16 q or k projection psum tile: [128 out, TWIN tokens]."""
        qps = psum.tile([P, TWIN], f32, name="qps", tag="qkv")
        # psum sub-region accumulation groups must not interleave in
        # emission order: keep the region (j) loop OUTER
        if by_block:
            for j in range(2):
                for k in range(CN):
                    lhsT = w[:, k, :] if wsel is None else w[:, wsel, k, :]
                    nc.tensor.matmul(
                        qps[:, XB * j: XB * (j + 1)],
                        lhsT=lhsT,
                        rhs=xT_sb[:, 2 * t + j, k, :],
                        start=(k == 0),
                        stop=(k == CN - 1),
                    )
        else:
            for k in range(CN):
                lhsT = w[:, k, :] if wsel is None else w[:, wsel, k, :]
                nc.tensor.matmul(
                    qps[:],
                    lhsT=lhsT,
                    rhs=xT_sb[:, 2 * t: 2 * t + 2, k, :],
                    start=(k == 0),
                    stop=(k == CN - 1),
                )
        dst = qkT_sb[:, CN * qk + hp, TWIN * t: TWIN * (t + 1)]
        if (qk + t) % 2 == 0:
            nc.vector.tensor_copy(dst, qps[:])
        else:
            nc.scalar.activation(dst, qps[:], AF.Copy)

    def emit_v(oh, m):
        """One v projection chunk: psum [128 tokens, 384 outs]."""
        vps = psum.tile([P, TWIN], f32, name="vps", tag="qkv")
        for k in range(CN):
            nc.tensor.matmul(
                vps[:, 0:384],
                lhsT=xT_sb[:, m // 2, k, P * (m % 2): P * (m % 2) + P],
                rhs=wv[:, oh, k, :],
                start=(k == 0),
                stop=(k == CN - 1),
            )
        nc.scalar.activation(
            V_sb[:, m, 6 * oh: 6 * (oh + 1), 0:64],
            vps[:, 0:384].rearrange("p (a b) -> p a b", b=64),
            AF.Copy,
        )

    def pull(filler):
        if filler:
            filler.pop(0)()

    def emit_attention(hp, filler):
        """S, softmax and P@V for head pair hp (both heads interleaved).

        filler: list of thunks emitting independent PE work; one is
        pulled after each S chunk-pair and each P@V group so the PE has
        fill work while the exp/mask chains run.
        """
        ets = [etpool.tile([P, MC, AWIN], bf16, name="et%d" % hh, tag="et")
               for hh in range(2)]
        for cp in range(MC // 2):
            spss = [spool_box["p"].tile([P, 2, TWIN], f32, name="sps%d" % hh,
                                        tag="sps")
                    for hh in range(2)]
            for j in range(2):
                c = 2 * cp + j
                s = WIN_START[c]
                for hh in range(2):
                    prange = slice(64 * hh, 64 * hh + 64)
                    nc.tensor.matmul(
                        spss[hh][:, j, 0:AWIN],
                        lhsT=qkT_sb[prange, CN + hp, P * c: P * (c + 1)],
                        rhs=qkT_sb[prange, hp, s: s + AWIN],
                        start=True,
                        stop=True,
                        tile_position=(64 * hh, 0),
                    )
            pull(filler)
            for hh in range(2):
                esc = expool.tile([P, 2, AWIN], bf16, name="esc", tag="esc")
                nc.scalar.activation(
                    esc[:], spss[hh][:, :, 0:AWIN], AF.Exp)
                nc.vector.tensor_mul(
                    ets[hh][:, 2 * cp: 2 * cp + 2, :].rearrange(
                        "p a b -> p (a b)"),
                    esc[:].rearrange("p a b -> p (a b)"),
                    bmask_sb[:, 2 * cp: 2 * cp + 2, :].rearrange(
                        "p a b -> p (a b)"),
                )
        if debug is not None:
            for hh in range(2):
                if ("d_et%d" % (2 * hp + hh)) in debug:
                    nc.sync.dma_start(debug["d_et%d" % (2 * hp + hh)][:], ets[hh][:])
        # P@V with ones column. hh=0: lhsT = [v,ones,...] -> O^T on psum
        # rows 0:64, r on row 64. hh=1: lhsT starts 64 cols earlier so
        # O^T lands on rows 64:128 with r on row 32 (the previous head's
        # ones column); the other rows hold garbage that is never read.
        # 128-wide weight loads keep FWL enabled for both.
        for t in range(NT):
            ots = []
            for hh in range(2):
                h = 2 * hp + hh
                et = ets[hh]
                ot = psum.tile([P, TWIN], f32, name="ot%d" % hh, tag="ot")
                ots.append(ot)
                base = VW * h if hh == 0 else VW * h - 64
                cs = [c for c in range(MC)
                      if min(WIN_START[c] + AWIN, TWIN * (t + 1)) > max(WIN_START[c], TWIN * t)]
                # widest-overlap chunk first so the start=True matmul covers
                # the largest psum range (per-element has_written then only
                # ever accumulates into written elements)
                cs.sort(key=lambda c: max(WIN_START[c], TWIN * t)
                        - min(WIN_START[c] + AWIN, TWIN * (t + 1)))
                for i, c in enumerate(cs):
                    lo = max(WIN_START[c], TWIN * t)
                    hi = min(WIN_START[c] + AWIN, TWIN * (t + 1))
                    nc.tensor.matmul(
                        ot[:, lo - TWIN * t: hi - TWIN * t],
                        lhsT=V_flat[:, c, base: base + 128],
                        rhs=et[:, c, lo - WIN_START[c]: hi - WIN_START[c]],
                        start=(i == 0),
                        stop=(i == len(cs) - 1),
                    )
            pull(filler)
            # normalize: both r rows -> reciprocal -> per-head broadcast ->
            # per-head multiply straight into OT_sb
            # stage the r rows to SBUF (reciprocal_approx_fast misreads
            # PSUM operands on HW), per-head fast reciprocal, then GPSIMD
            # partition broadcast (output must start at partition 0)
            rr = rpool.tile([1, 2, TWIN], f32, name="rr", tag="rr")
            nc.scalar.activation(rr[:, 0, :], ots[0][64:65, :], AF.Copy)
            nc.vector.tensor_copy(rr[:, 1, :], ots[1][32:33, :])
            dst0 = OT_sb[0:64, hp, TWIN * t: TWIN * (t + 1)]
            dst1 = OT_sb[64:128, hp, TWIN * t: TWIN * (t + 1)]
            invr = rpool.tile([1, 2, TWIN], f32, name="invr", tag="invr")
            invb0 = bpool.tile([P, TWIN], f32, name="invb0", tag="invb")
            invb1 = bpool.tile([P, TWIN], f32, name="invb1", tag="invb")
            nc.vector.reciprocal_approx_fast(invr[:, 0, :], rr[:, 0, :])
            nc.gpsimd.partition_broadcast(invb0[:, :], invr[:, 0, :])
            nc.vector.reciprocal_approx_fast(invr[:, 1, :], rr[:, 1, :])
            nc.gpsimd.partition_broadcast(invb1[:, :], invr[:, 1, :])
            nc.vector.tensor_mul(dst0, ots[0][0:64, :], invb0[0:64, :])
            nc.vector.tensor_mul(dst1, ots[1][64:128, :], invb1[64:128, :])

    # Output projection partials: k-chunks accumulate into an SBUF
    # partial as soon as the corresponding attention pairs finish; the
    # tail folds the remaining chunks + bias + partial per (oc, t).
    # Partials only use chunks 0..3 so they are safe fillers one slot
    # later (a filler must never depend on the current pair's output -
    # it would stall the in-order PE queue).
    KSPLIT = {0: 4, 1: 4}
    partial_sb = consts.tile([P, CN, N], bf16)

    def emit_proj_part(t, oc):
        pps = psum.tile([P, TWIN], f32, name="pps", tag="qkv")
        for k in range(KSPLIT[t]):
            nc.tensor.matmul(
                pps[:],
                lhsT=wprojT_sb[:, k, P * oc: P * (oc + 1)],
                rhs=OT_sb[:, k, TWIN * t: TWIN * (t + 1)],
                start=(k == 0),
                stop=(k == KSPLIT[t] - 1),
            )
        dst = partial_sb[:, oc, TWIN * t: TWIN * (t + 1)]
        if oc % 2 == 0:
            nc.vector.tensor_copy(dst, pps[:])
        else:
            nc.scalar.activation(dst, pps[:], AF.Copy)

    # Attention runs pairs in order [0,1,2,3,5,4] so both projection
    # partials (k-chunks 0..3) become safe fillers for the last two
    # positions. Fillers per position: next pair's q (bf16) + k (fp8)
    # projections, v head-triples (g2 = heads 6-8 before position 3,
    # g3 = heads 9-11 before position 4), then the partials.
    ATT_ORDER = [0, 1, 2, 3, 4, 5]
    V1_MS = {0: [0, 1, 2], 1: [3, 4, 5], 2: [6, 7]}

    with tc.tile_pool(name="spsp", bufs=2, space="PSUM") as spool:
        spool_box["p"] = spool
        # pair-0 q/k projection + first-half v: the PE's warm-up block
        for t in range(NT):
            emit_qk(0, wqk0, 0, t, by_block=True, wsel=0)
        for t in range(NT):
            emit_qk(0, wqk0, 1, t, by_block=True, wsel=1)
        for m in range(MC):
            emit_v(0, m)
        wq_next = load_wqk(1)
        for pos in range(CN):
            hp = ATT_ORDER[pos]
            wq_cur = wq_next
            filler = []
            if pos + 1 < CN:
                hpn = ATT_ORDER[pos + 1]
                if pos + 2 < CN:
                    wq_next = load_wqk(ATT_ORDER[pos + 2])
                for qk in range(2):
                    for t in range(NT):
                        filler.append(
                            lambda hpn=hpn, qk=qk, t=t, w=wq_cur: emit_qk(
                                hpn, w, qk, t, by_block=False, wsel=qk))
            if pos in V1_MS:
                for m in V1_MS[pos]:
                    filler.append(lambda m=m: emit_v(1, m))
            if pos == 5:
                # both partials fill the last pair's softmax chains; the
                # leftovers bridge the window between the last P@V and
                # the projection tail
                for oc in range(CN):
                    filler.append(lambda oc=oc: emit_proj_part(0, oc))
                    filler.append(lambda oc=oc: emit_proj_part(1, oc))
            emit_attention(hp, filler)
            for f in filler:
                f()

    # ---------------- output projection tail ----------------
    # t=0 first: its OT chunks complete before t=1's (the t=1 chains of
    # the last pair are the final attention work)
    with tc.tile_pool(name="outst", bufs=4) as ostpool, \
            tc.tile_pool(name="tailp", bufs=4, space="PSUM") as tpool:
        # Per t, issue the k=4 matmuls (which depend only on pair 4's
        # output) for as many ocs as psum buffers allow BEFORE any k=5
        # matmul: a k=5 waits on the last pair's softmax chain, and in
        # the in-order PE queue it would otherwise block the independent
        # k=4 work behind it.
        PRE = 4
        for t in range(NT):
            def k4(oc, t=t):
                pps = tpool.tile([P, TWIN], f32, name="pps", tag="tpps")
                nc.tensor.matmul(
                    pps[:],
                    lhsT=wprojT_sb[:, 4, P * oc: P * (oc + 1)],
                    rhs=OT_sb[:, 4, TWIN * t: TWIN * (t + 1)],
                    start=True,
                    stop=False,
                )
                return pps
            pend = {oc: k4(oc) for oc in range(PRE)}
            for oc in range(CN):
                pps = pend.pop(oc)
                nc.tensor.matmul(
                    pps[:],
                    lhsT=wprojT_sb[:, 5, P * oc: P * (oc + 1)],
                    rhs=OT_sb[:, 5, TWIN * t: TWIN * (t + 1)],
                    start=False,
                    stop=True,
                )
                ost = ostpool.tile([P, TWIN], bf16, name="ost", tag="ost")
                # ost = (pps + bias) + partial in one DVE op
                nc.vector.scalar_tensor_tensor(
                    ost[:], pps[:], bias_sb[:, oc: oc + 1],
                    partial_sb[:, oc, TWIN * t: TWIN * (t + 1)],
                    add, add)
                nc.sync.dma_start(
                    outT[P * oc: P * (oc + 1), TWIN * t: TWIN * (t + 1)], ost[:])
                if PRE + oc < CN:
                    pend[PRE + oc] = k4(PRE + oc)


def _build(debug_shapes=False):
    global _PROG
    if _PROG is not None:
        return _PROG
    from contextlib import ExitStack

    from concourse import bacc
    import concourse.mybir as mybir
    import concourse.tile as tile

    f32 = mybir.dt.float32
    bf16 = mybir.dt.bfloat16

    nc = bacc.Bacc("TRN2", target_bir_lowering=False, debug=False,
                   enable_asserts=False)
    xT = nc.dram_tensor("xT", [P, CN * N], bf16, kind="ExternalInput").ap()
    wqkvT = nc.dram_tensor("wqkvT", [P, WQKV_COLS], bf16, kind="ExternalInput").ap()
    wprojT = nc.dram_tensor("wprojT", [P, CN * DIM], bf16, kind="ExternalInput").ap()
    biasT = nc.dram_tensor("biasT", [P, CN], f32, kind="ExternalInput").ap()
    bmask = nc.dram_tensor("bmask", [P, MC, AWIN], bf16, kind="ExternalInput").ap()
    ident = nc.dram_tensor("ident", [P, P], bf16, kind="ExternalInput").ap()
    outT = nc.dram_tensor("outT", [DIM, N], bf16, kind="ExternalOutput").ap()

    with tile.TileContext(nc) as tc:
        with ExitStack() as ctx:
            _emit(ctx, tc, (xT, wqkvT, wprojT, biasT, bmask, ident, outT))
    nc.compile()
    _PROG = nc
    return nc


def _host_inputs(x, w_qkv, w_proj, b_proj, mask):
    import ml_dtypes

    x = np.asarray(x, dtype=np.float32)
    w_qkv = np.asarray(w_qkv, dtype=np.float32)
    w_proj = np.asarray(w_proj, dtype=np.float32)
    b_proj = np.asarray(b_proj, dtype=np.float32)
    mask = np.asarray(mask, dtype=np.float32)

    wq = w_qkv.copy()
    wq[0:DIM] *= SCALE
    wT = np.ascontiguousarray(wq.T)                          # [768 in, 2304 out]
    # q/k blocks: [p, hp, qk, co, 128] ; v blocks: [p, oh, co, 384]
    wqkv_host = np.empty((P, WQKV_COLS), dtype=np.float32)
    for hp in range(CN):
        for qk in range(2):
            blk = wT[:, DIM * qk + P * hp: DIM * qk + P * hp + P]  # [768, 128]
            blk = blk.reshape(CN, P, P).transpose(1, 0, 2).reshape(P, CN * P)
            base = (2 * hp + qk) * CN * P
            wqkv_host[:, base: base + CN * P] = blk
    for oh in range(2):
        blk = wT[:, 2 * DIM + 384 * oh: 2 * DIM + 384 * (oh + 1)]  # [768, 384]
        blk = blk.reshape(CN, P, 384).transpose(1, 0, 2).reshape(P, CN * 384)
        wqkv_host[:, WV_OFF + oh * CN * 384: WV_OFF + (oh + 1) * CN * 384] = blk
    wqkvT = wqkv_host.astype(ml_dtypes.bfloat16)

    wprojT = np.ascontiguousarray(
        w_proj.T.reshape(CN, P, DIM).transpose(1, 0, 2).reshape(P, CN * DIM)
    ).astype(ml_dtypes.bfloat16)
    biasT = np.ascontiguousarray(b_proj.reshape(CN, P).T)    # [128, 6]

    vis = (mask[0, 0] == 0.0)
    vis_w = vis[np.ix_(PERM, PERM)]
    bm = np.zeros((P, MC, AWIN), dtype=ml_dtypes.bfloat16)
    for c in range(MC):
        s = WIN_START[c]
        bm[:, c, :] = vis_w[c * P:(c + 1) * P, s: s + AWIN]

    in_maps = []
    for b in range(B):
        xw = np.ascontiguousarray(x[b].T[:, PERM])           # [768, 1024] w-major
        # bf16: [p, block, c, 256]
        xTb = (xw.reshape(CN, P, NXB, XB).transpose(1, 2, 0, 3)
               .reshape(P, CN * N))
        in_maps.append({
            "xT": xTb.astype(ml_dtypes.bfloat16),
            "wqkvT": wqkvT,
            "wprojT": wprojT,
            "biasT": biasT,
            "bmask": bm,
            "ident": np.eye(P, dtype=ml_dtypes.bfloat16),
        })
    return in_maps


PROFILE = False
LAST_RESULT = None


def kernel(x, w_qkv, w_proj, b_proj, mask):
    global LAST_RESULT
    from concourse.bass_utils import run_bass_kernel_spmd

    nc = _build()
    in_maps = _host_inputs(x, w_qkv, w_proj, b_proj, mask)
    res = run_bass_kernel_spmd(nc, in_maps, core_ids=list(range(B)),
                               trace=PROFILE)
    LAST_RESULT = res
    out = np.empty((B, N, DIM), dtype=np.float32)
    for b in range(B):
        out[b][PERM, :] = np.asarray(res.results[b]["outT"]).astype(np.float32).T
    return np.ascontiguousarray(out)
